# revision 33
# baseline (speedup 1.0000x reference)
"""DSAFT NKSPL loss on 8 Trainium2 cores.

Math (faithful to the reference):
    theta = log_h.ravel();  e = log(durations + 1e-32) - theta
    perm = argsort(e); e_sorted = e[perm]; inv = argsort(perm)
    ev = events[inv]; th = theta[inv]
    cond_E_i = sum_j exp(-(e_i-e_j)^2/2) * ev_j / (n*sqrt(2pi)) + n*1e-32
    surv_i   = 0.5 + sum_j erf((e_i-e_j)/sqrt(2)) / (2n)
    loss = -sum_i (log cond_E_i - log surv_i + th_i) * ev_i / n

Only rows with ev_i == 1 contribute, and only columns with ev_j == 1
contribute to cond_E, so the device only computes the n1 (~n/2) event
rows: pdf row-sums over the gathered event columns via one
Derivative_Erf activation per 128-row chunk (derivative_erf(u) =
2/sqrt(pi)*exp(-u^2), u = (e_i-e_j)/sqrt(2); free affine via scale +
per-partition bias; row-sum via accum_out) and erf row-sums over all
n columns via one Erf activation per chunk.

Sharding: the first K*1024 event rows go to the 8 cores as contiguous
blocks of K full 128-row chunks.  The remaining <1024 rows form shared
chunks whose COLUMNS are split 8 ways across cores (so no core pays a
full-width pass for a mostly-padded chunk); the host sums those
per-core partial accumulators and folds in the remainder rows' loss
terms.  Each core reduces its full chunks to a partial scalar
(log/mask/reduce epilogue on-device); the host adds the 8 partials.
"""

import math
from contextlib import ExitStack

import numpy as np

from bass_rust import add_dep_helper
from concourse import bacc, mybir, tile
from concourse.bass_utils import run_bass_kernel_spmd
from concourse.tile_utils import partition_sum

N_CORES = 8
P = 128
_EPS = 1e-32
RSQRT2 = 1.0 / math.sqrt(2.0)
PAD_COL = 1.0e3  # far from every real e value

# program cache keyed by (k_full, c_sh, ne, na_pad, n)
_nc_cache: dict[tuple, object] = {}
# results of the last run_bass_kernel_spmd call (for the test harness)
LAST_RESULTS = None
TRACE = False


def _build(k_full: int, c_sh: int, ne: int, na: int, n: int):
    """Build the per-core Bass program.

    k_full: full 128-row chunks per core;  c_sh: shared chunks (columns
    split across cores);  ne: padded event-column count;  na: padded
    total column count;  n: true problem size (normalizers).
    """
    nc = bacc.Bacc(None, target_bir_lowering=False)
    ne_nar = ne // N_CORES
    na_nar = na // N_CORES
    nch = k_full + c_sh

    e_ev = nc.dram_tensor("e_ev", [ne], mybir.dt.float32, kind="ExternalInput")
    e_all = nc.dram_tensor("e_all", [na], mybir.dt.float32, kind="ExternalInput")
    if c_sh:
        e_ev_nar = nc.dram_tensor(
            "e_ev_nar", [ne_nar], mybir.dt.float32, kind="ExternalInput"
        )
        e_all_nar = nc.dram_tensor(
            "e_all_nar", [na_nar], mybir.dt.float32, kind="ExternalInput"
        )
    # bias values e_row/sqrt(2): [shared chunk rows..., own full rows...]
    er_b = nc.dram_tensor("er_b", [nch * P], mybir.dt.float32, kind="ExternalInput")

    partial = nc.dram_tensor("partial", [1, 1], mybir.dt.float32, kind="ExternalOutput")
    if c_sh:
        rowacc = nc.dram_tensor(
            "rowacc", [2, P, c_sh], mybir.dt.float32, kind="ExternalOutput"
        )

    k1 = 1.0 / (2.0 * math.sqrt(2.0) * n)  # pdf raw-sum -> cond_E
    c0 = n * _EPS
    k2 = 1.0 / (2.0 * n)  # erf raw-sum -> surv
    # padded e_all columns each contribute erf((e_i - 1e3)/sqrt2) = -1
    half_eff = 0.5 + k2 * (na - n)

    with tile.TileContext(nc) as tc, ExitStack() as ctx:
        const = ctx.enter_context(tc.tile_pool(name="const", bufs=1))
        scratch = ctx.enter_context(tc.tile_pool(name="scratch", bufs=1))
        acc = ctx.enter_context(tc.tile_pool(name="acc", bufs=1))

        # a first ACT op with no input dependencies lets bacc place the
        # derivative_erf table load while the input DMAs are in flight
        dmy = const.tile([P, 1], mybir.dt.float32)
        nc.vector.memset(dmy[:], 0.0)
        dummy_act = nc.scalar.activation(
            dmy[:], dmy[:], mybir.ActivationFunctionType.Derivative_Erf
        )

        # the first ACT ops are the chunk-0 pdf quarters: they need erb_t
        # plus successive ebc_ev column slices, so those DMAs go first
        erb_t = const.tile([P, nch], mybir.dt.float32)
        nc.sync.dma_start(erb_t[:], er_b[:].rearrange("(c p) -> p c", p=P))

        # event-column vector replicated across all 128 partitions, loaded
        # in column slices so the pdf quarters can start early; SWDGE
        # (gpsimd) queues keep descriptor generation off the sync
        # sequencer's critical path
        ebc_ev = const.tile([P, ne], mybir.dt.float32)
        ebc_all = const.tile([P, na], mybir.dt.float32)
        ev_dmas = []
        ncs = 4
        cstep = ne // ncs
        for s in range(ncs):
            ev_dmas.append(
                nc.gpsimd.dma_start(
                    ebc_ev[:, s * cstep : (s + 1) * cstep],
                    e_ev[s * cstep : (s + 1) * cstep][None, :].to_broadcast(
                        (P, cstep)
                    ),
                )
            )

        # everything below is needed only later in the pdf phase; keep it
        # off the ebc_ev critical path (sequencer order + explicit deps)
        if c_sh:
            ebc_ev_nar = const.tile([P, ne_nar], mybir.dt.float32)
            nc.gpsimd.dma_start(
                ebc_ev_nar[:], e_ev_nar[None, :].to_broadcast((P, ne_nar))
            )
            ebc_all_nar = const.tile([P, na_nar], mybir.dt.float32)
            nar_dma = nc.sync.dma_start(
                ebc_all_nar[:], e_all_nar[None, :].to_broadcast((P, na_nar))
            )
            add_dep_helper(nar_dma.ins, ev_dmas[-1].ins, sync=True,
                           reason="ebc_all_nar after ebc_ev")

        n_split = 8
        step = P // n_split
        for s in range(n_split):
            all_dma = nc.sync.dma_start(
                ebc_all[s * step : (s + 1) * step, :],
                e_all[None, :].to_broadcast((step, na)),
            )
            # the pdf phase needs ebc_ev first; don't let these large
            # transfers contend with it for DMA bandwidth
            add_dep_helper(
                all_dma.ins,
                ev_dmas[-1].ins,
                sync=True,
                reason="ebc_all after ebc_ev (DMA bandwidth ordering)",
            )

        # One shared scratch output for every ACT op: the overlapping
        # writes (WAW) pin the scalar-engine order to emission order, so
        # each activation table set loads exactly once
        # (erf_derivative -> sigmoid_and_others -> natural_log).
        out_scr = scratch.tile([P, na], mybir.dt.float32)
        # accum slots: shared chunks first, then own full chunks
        acc_pdf = acc.tile([P, nch], mybir.dt.float32)
        acc_erf = acc.tile([P, nch], mybir.dt.float32)

        first_real = None
        # first full pdf chunk runs as column-quarters aligned with the
        # ebc_ev column slices, so ACT works while the slices stream in
        acc_h = None
        if k_full:
            acc_h = acc.tile([P, ncs], mybir.dt.float32, tag="acc_h")
        for c in range(k_full):
            if c == 0:
                prev = None
                for s in range(ncs):
                    q = nc.scalar.activation(
                        out_scr[:, s * cstep : (s + 1) * cstep],
                        ebc_ev[:, s * cstep : (s + 1) * cstep],
                        mybir.ActivationFunctionType.Derivative_Erf,
                        bias=erb_t[:, c_sh : c_sh + 1],
                        scale=-RSQRT2,
                        accum_out=acc_h[:, s : s + 1],
                    )
                    # disjoint out_scr slices carry no WAW edge; pin order
                    if prev is not None:
                        add_dep_helper(q.ins, prev.ins, sync=False,
                                       reason="pdf quarter order")
                    prev = q
                    if first_real is None:
                        first_real = q
                nc.vector.reduce_sum(
                    acc_pdf[:, c_sh : c_sh + 1],
                    acc_h[:],
                    axis=mybir.AxisListType.X,
                )
                continue
            nc.scalar.activation(
                out_scr[:, :ne],
                ebc_ev[:],
                mybir.ActivationFunctionType.Derivative_Erf,
                bias=erb_t[:, c_sh + c : c_sh + c + 1],
                scale=-RSQRT2,
                accum_out=acc_pdf[:, c_sh + c : c_sh + c + 1],
            )
        # narrow (shared-chunk) pdf ops go AFTER the full chunks: their
        # per-core input arrives later than the ebc_ev slices
        for c in range(c_sh):
            a = nc.scalar.activation(
                out_scr[:, :ne_nar],
                ebc_ev_nar[:],
                mybir.ActivationFunctionType.Derivative_Erf,
                bias=erb_t[:, c : c + 1],
                scale=-RSQRT2,
                accum_out=acc_pdf[:, c : c + 1],
            )
            if first_real is None:
                first_real = a
        for c in range(c_sh):
            nc.scalar.activation(
                out_scr[:, :na_nar],
                ebc_all_nar[:],
                mybir.ActivationFunctionType.Erf,
                bias=erb_t[:, c : c + 1],
                scale=-RSQRT2,
                accum_out=acc_erf[:, c : c + 1],
            )
        for c in range(k_full):
            nc.scalar.activation(
                out_scr[:, :na],
                ebc_all[:],
                mybir.ActivationFunctionType.Erf,
                bias=erb_t[:, c_sh + c : c_sh + c + 1],
                scale=-RSQRT2,
                accum_out=acc_erf[:, c_sh + c : c_sh + c + 1],
            )

        # the dummy op must precede all real ACT work on the engine
        add_dep_helper(first_real.ins, dummy_act.ins, sync=False,
                       reason="table-load hoist dummy first")

        if c_sh:
            nc.sync.dma_start(rowacc[0], acc_pdf[:, :c_sh])
            nc.sync.dma_start(rowacc[1], acc_erf[:, :c_sh])

        if k_full:
            # log(cond_E) and log(surv) for own full chunks; outputs into
            # out_scr slices keep the WAW chain so the Ln ops schedule
            # after the erf ops.  (the th term is assembled host-side)
            c0_t = acc.tile([P, 1], mybir.dt.float32)
            nc.vector.memset(c0_t[:], c0)
            half_t = acc.tile([P, 1], mybir.dt.float32)
            nc.vector.memset(half_t[:], half_eff)
            ln_ce = out_scr[:, 0:k_full]
            ln_sv = out_scr[:, k_full : 2 * k_full]
            nc.scalar.activation(
                ln_ce,
                acc_pdf[:, c_sh:],
                mybir.ActivationFunctionType.Ln,
                bias=c0_t[:],
                scale=k1,
            )
            nc.scalar.activation(
                ln_sv,
                acc_erf[:, c_sh:],
                mybir.ActivationFunctionType.Ln,
                bias=half_t[:],
                scale=k2,
            )

            # row_l[p] = sum_c (ln_ce - ln_sv); then -1/n * partition-sum
            t1 = acc.tile([P, k_full], mybir.dt.float32)
            nc.vector.tensor_sub(t1[:], ln_ce, ln_sv)
            row_l = acc.tile([P, 1], mybir.dt.float32)
            nc.vector.reduce_sum(row_l[:], t1[:], axis=mybir.AxisListType.X)
            tot = acc.tile([1, 1], mybir.dt.float32)
            partition_sum(tc, tot[:1, :1], row_l[:, :1])
            res = acc.tile([1, 1], mybir.dt.float32)
            nc.scalar.mul(res[:1, :1], tot[:1, :1], -1.0 / n)
            nc.sync.dma_start(partial[:], res[:1, :1])
        else:
            res = acc.tile([1, 1], mybir.dt.float32)
            nc.vector.memset(res[:1, :1], 0.0)
            nc.sync.dma_start(partial[:], res[:1, :1])

    nc.compile()
    return nc


def kernel(log_h: np.ndarray, durations: np.ndarray, events: np.ndarray) -> np.ndarray:
    global LAST_RESULTS

    theta = np.asarray(log_h).astype(np.float32, copy=False).reshape(-1)
    durations = np.asarray(durations).astype(np.float32, copy=False)
    events = np.asarray(events)
    n = int(theta.shape[0])

    e = -(theta - np.log(durations + np.float32(_EPS)))
    perm = np.argsort(e, kind="stable")
    e_sorted = np.ascontiguousarray(e[perm])
    inv = np.argsort(perm, kind="stable")
    ev = events.astype(np.float32)[inv]
    th_s = theta[inv]

    idx = np.nonzero(ev > 0.5)[0]
    n1 = int(idx.size)
    if n1 == 0:
        return np.array(-0.0, dtype=np.float32)

    e1 = e_sorted[idx]
    th1 = th_s[idx]

    ne = -(-n1 // P) * P  # event cols padded to a multiple of 128
    na = -(-n // N_CORES) * N_CORES  # total cols padded to a multiple of 8
    k_full = n1 // (P * N_CORES)  # full 128-row chunks per core
    rem = n1 - k_full * P * N_CORES
    c_sh = -(-rem // P)  # shared chunks, columns split across cores
    nch = k_full + c_sh
    ne_nar = ne // N_CORES
    na_nar = na // N_CORES

    e_ev = np.full(ne, PAD_COL, dtype=np.float32)
    e_ev[:n1] = e1
    e_all = np.full(na, PAD_COL, dtype=np.float32)
    e_all[:n] = e_sorted

    # shared-chunk bias rows (identical on every core)
    shared_b = np.zeros(c_sh * P, dtype=np.float32)
    n_shared = rem
    if n_shared:
        shared_b[:n_shared] = e1[k_full * P * N_CORES :] * np.float32(RSQRT2)

    in_maps = []
    for c in range(N_CORES):
        s = c * k_full * P
        erb = np.empty(nch * P, dtype=np.float32)
        erb[: c_sh * P] = shared_b
        erb[c_sh * P :] = e1[s : s + k_full * P] * np.float32(RSQRT2)
        m = {"e_ev": e_ev, "e_all": e_all, "er_b": erb}
        if c_sh:
            m["e_ev_nar"] = np.ascontiguousarray(
                e_ev[c * ne_nar : (c + 1) * ne_nar]
            )
            m["e_all_nar"] = np.ascontiguousarray(
                e_all[c * na_nar : (c + 1) * na_nar]
            )
        in_maps.append(m)

    key = (k_full, c_sh, ne, na, n)
    if key not in _nc_cache:
        _nc_cache[key] = _build(*key)
    nc = _nc_cache[key]

    LAST_RESULTS = run_bass_kernel_spmd(
        nc, in_maps, core_ids=list(range(N_CORES)), trace=TRACE
    )

    loss = np.float64(0.0)
    for r in LAST_RESULTS.results:
        loss += float(r["partial"].reshape(()))
    # th term for the full-slot rows (device partials carry only the logs)
    loss += -np.float64(np.sum(th1[: k_full * P * N_CORES], dtype=np.float64)) / n

    if c_sh:
        # combine the shared chunks' per-core partial accumulators
        praw = np.zeros((P, c_sh), dtype=np.float64)
        eraw = np.zeros((P, c_sh), dtype=np.float64)
        for r in LAST_RESULTS.results:
            praw += r["rowacc"][0].astype(np.float64)
            eraw += r["rowacc"][1].astype(np.float64)
        praw = praw.T.reshape(-1)[:n_shared]  # rows are (c p)
        eraw = eraw.T.reshape(-1)[:n_shared]
        cond_e = praw / (2.0 * math.sqrt(2.0) * n) + n * _EPS
        surv = 0.5 + (eraw + (na - n)) / (2.0 * n)
        th_sh = th1[k_full * P * N_CORES :].astype(np.float64)
        loss += -np.sum(np.log(cond_e) - np.log(surv) + th_sh) / n

    return np.asarray(loss, dtype=np.float32)


# revision 34
# speedup vs baseline: 1.0045x; 1.0045x over previous
"""DSAFT NKSPL loss on 8 Trainium2 cores.

Math (faithful to the reference):
    theta = log_h.ravel();  e = log(durations + 1e-32) - theta
    perm = argsort(e); e_sorted = e[perm]; inv = argsort(perm)
    ev = events[inv]; th = theta[inv]
    cond_E_i = sum_j exp(-(e_i-e_j)^2/2) * ev_j / (n*sqrt(2pi)) + n*1e-32
    surv_i   = 0.5 + sum_j erf((e_i-e_j)/sqrt(2)) / (2n)
    loss = -sum_i (log cond_E_i - log surv_i + th_i) * ev_i / n

Only rows with ev_i == 1 contribute, and only columns with ev_j == 1
contribute to cond_E, so the device only computes the n1 (~n/2) event
rows: pdf row-sums over the gathered event columns via one
Derivative_Erf activation per 128-row chunk (derivative_erf(u) =
2/sqrt(pi)*exp(-u^2), u = (e_i-e_j)/sqrt(2); free affine via scale +
per-partition bias; row-sum via accum_out) and erf row-sums over all
n columns via one Erf activation per chunk.

Sharding: the first K*1024 event rows go to the 8 cores as contiguous
blocks of K full 128-row chunks.  The remaining <1024 rows form shared
chunks whose COLUMNS are split 8 ways across cores (so no core pays a
full-width pass for a mostly-padded chunk); the host sums those
per-core partial accumulators and folds in the remainder rows' loss
terms.  Each core reduces its full chunks to a partial scalar
(log/mask/reduce epilogue on-device); the host adds the 8 partials.
"""

import math
from contextlib import ExitStack

import numpy as np

from bass_rust import add_dep_helper
from concourse import bacc, mybir, tile
from concourse.bass_utils import run_bass_kernel_spmd
from concourse.tile_utils import partition_sum

N_CORES = 8
P = 128
_EPS = 1e-32
RSQRT2 = 1.0 / math.sqrt(2.0)
PAD_COL = 1.0e3  # far from every real e value

# program cache keyed by (k_full, c_sh, ne, na_pad, n)
_nc_cache: dict[tuple, object] = {}
# results of the last run_bass_kernel_spmd call (for the test harness)
LAST_RESULTS = None
TRACE = False


def _build(k_full: int, c_sh: int, ne: int, na: int, n: int):
    """Build the per-core Bass program.

    k_full: full 128-row chunks per core;  c_sh: shared chunks (columns
    split across cores);  ne: padded event-column count;  na: padded
    total column count;  n: true problem size (normalizers).
    """
    nc = bacc.Bacc(None, target_bir_lowering=False)
    ne_nar = ne // N_CORES
    na_nar = na // N_CORES
    nch = k_full + c_sh

    e_ev = nc.dram_tensor("e_ev", [ne], mybir.dt.float32, kind="ExternalInput")
    e_all = nc.dram_tensor("e_all", [na], mybir.dt.float32, kind="ExternalInput")
    if c_sh:
        e_ev_nar = nc.dram_tensor(
            "e_ev_nar", [ne_nar], mybir.dt.float32, kind="ExternalInput"
        )
        e_all_nar = nc.dram_tensor(
            "e_all_nar", [na_nar], mybir.dt.float32, kind="ExternalInput"
        )
    # bias values e_row/sqrt(2): [shared chunk rows..., own full rows...]
    er_b = nc.dram_tensor("er_b", [nch * P], mybir.dt.float32, kind="ExternalInput")

    partial = nc.dram_tensor("partial", [1, 1], mybir.dt.float32, kind="ExternalOutput")
    if c_sh:
        rowacc = nc.dram_tensor(
            "rowacc", [2, P, c_sh], mybir.dt.float32, kind="ExternalOutput"
        )

    k1 = 1.0 / (2.0 * math.sqrt(2.0) * n)  # pdf raw-sum -> cond_E
    c0 = n * _EPS
    k2 = 1.0 / (2.0 * n)  # erf raw-sum -> surv
    # padded e_all columns each contribute erf((e_i - 1e3)/sqrt2) = -1
    half_eff = 0.5 + k2 * (na - n)

    with tile.TileContext(nc) as tc, ExitStack() as ctx:
        const = ctx.enter_context(tc.tile_pool(name="const", bufs=1))
        scratch = ctx.enter_context(tc.tile_pool(name="scratch", bufs=1))
        acc = ctx.enter_context(tc.tile_pool(name="acc", bufs=1))

        # a first ACT op with no input dependencies lets bacc place the
        # derivative_erf table load while the input DMAs are in flight
        dmy = const.tile([P, 1], mybir.dt.float32)
        nc.vector.memset(dmy[:], 0.0)
        dummy_act = nc.scalar.activation(
            dmy[:], dmy[:], mybir.ActivationFunctionType.Derivative_Erf
        )

        # the first ACT ops are the chunk-0 pdf quarters: they need erb_t
        # plus successive ebc_ev column slices, so those DMAs go first
        erb_t = const.tile([P, nch], mybir.dt.float32)
        nc.sync.dma_start(erb_t[:], er_b[:].rearrange("(c p) -> p c", p=P))

        # event-column vector replicated across all 128 partitions, loaded
        # in column slices so the pdf quarters can start early; SWDGE
        # (gpsimd) queues keep descriptor generation off the sync
        # sequencer's critical path
        ebc_ev = const.tile([P, ne], mybir.dt.float32)
        ebc_all = const.tile([P, na], mybir.dt.float32)
        ev_dmas = []
        ncs = 4
        cstep = ne // ncs
        for s in range(ncs):
            ev_dmas.append(
                nc.gpsimd.dma_start(
                    ebc_ev[:, s * cstep : (s + 1) * cstep],
                    e_ev[s * cstep : (s + 1) * cstep][None, :].to_broadcast(
                        (P, cstep)
                    ),
                )
            )

        # everything below is needed only later in the pdf phase; keep it
        # off the ebc_ev critical path (sequencer order + explicit deps)
        if c_sh:
            ebc_ev_nar = const.tile([P, ne_nar], mybir.dt.float32)
            nc.gpsimd.dma_start(
                ebc_ev_nar[:], e_ev_nar[None, :].to_broadcast((P, ne_nar))
            )
            ebc_all_nar = const.tile([P, na_nar], mybir.dt.float32)
            nar_dma = nc.sync.dma_start(
                ebc_all_nar[:], e_all_nar[None, :].to_broadcast((P, na_nar))
            )
            add_dep_helper(nar_dma.ins, ev_dmas[-1].ins, sync=True,
                           reason="ebc_all_nar after ebc_ev")

        n_split = 8
        step = P // n_split
        for s in range(n_split):
            all_dma = nc.sync.dma_start(
                ebc_all[s * step : (s + 1) * step, :],
                e_all[None, :].to_broadcast((step, na)),
            )
            # the pdf phase needs ebc_ev first; don't let these large
            # transfers contend with it for DMA bandwidth
            add_dep_helper(
                all_dma.ins,
                ev_dmas[-1].ins,
                sync=True,
                reason="ebc_all after ebc_ev (DMA bandwidth ordering)",
            )

        # One shared scratch output for every ACT op: the overlapping
        # writes (WAW) pin the scalar-engine order to emission order, so
        # each activation table set loads exactly once
        # (erf_derivative -> sigmoid_and_others -> natural_log).
        out_scr = scratch.tile([P, na], mybir.dt.float32)
        # accum slots: shared chunks first, then own full chunks
        acc_pdf = acc.tile([P, nch], mybir.dt.float32)
        acc_erf = acc.tile([P, nch], mybir.dt.float32)

        first_real = None
        # first full pdf chunk runs as column-quarters aligned with the
        # ebc_ev column slices, so ACT works while the slices stream in
        acc_h = None
        if k_full:
            acc_h = acc.tile([P, ncs], mybir.dt.float32, tag="acc_h")
        for c in range(k_full):
            if c == 0:
                prev = None
                for s in range(ncs):
                    q = nc.scalar.activation(
                        out_scr[:, s * cstep : (s + 1) * cstep],
                        ebc_ev[:, s * cstep : (s + 1) * cstep],
                        mybir.ActivationFunctionType.Derivative_Erf,
                        bias=erb_t[:, c_sh : c_sh + 1],
                        scale=-RSQRT2,
                        accum_out=acc_h[:, s : s + 1],
                    )
                    # disjoint out_scr slices carry no WAW edge; pin order
                    if prev is not None:
                        add_dep_helper(q.ins, prev.ins, sync=False,
                                       reason="pdf quarter order")
                    prev = q
                    if first_real is None:
                        first_real = q
                nc.vector.reduce_sum(
                    acc_pdf[:, c_sh : c_sh + 1],
                    acc_h[:],
                    axis=mybir.AxisListType.X,
                )
                continue
            nc.scalar.activation(
                out_scr[:, :ne],
                ebc_ev[:],
                mybir.ActivationFunctionType.Derivative_Erf,
                bias=erb_t[:, c_sh + c : c_sh + c + 1],
                scale=-RSQRT2,
                accum_out=acc_pdf[:, c_sh + c : c_sh + c + 1],
            )
        # narrow (shared-chunk) pdf ops go AFTER the full chunks: their
        # per-core input arrives later than the ebc_ev slices
        for c in range(c_sh):
            a = nc.scalar.activation(
                out_scr[:, :ne_nar],
                ebc_ev_nar[:],
                mybir.ActivationFunctionType.Derivative_Erf,
                bias=erb_t[:, c : c + 1],
                scale=-RSQRT2,
                accum_out=acc_pdf[:, c : c + 1],
            )
            if first_real is None:
                first_real = a
        for c in range(c_sh):
            nc.scalar.activation(
                out_scr[:, :na_nar],
                ebc_all_nar[:],
                mybir.ActivationFunctionType.Erf,
                bias=erb_t[:, c : c + 1],
                scale=-RSQRT2,
                accum_out=acc_erf[:, c : c + 1],
            )
        for c in range(k_full):
            nc.scalar.activation(
                out_scr[:, :na],
                ebc_all[:],
                mybir.ActivationFunctionType.Erf,
                bias=erb_t[:, c_sh + c : c_sh + c + 1],
                scale=-RSQRT2,
                accum_out=acc_erf[:, c_sh + c : c_sh + c + 1],
            )

        # the dummy op must precede all real ACT work on the engine
        add_dep_helper(first_real.ins, dummy_act.ins, sync=False,
                       reason="table-load hoist dummy first")

        if c_sh:
            nc.sync.dma_start(rowacc[0], acc_pdf[:, :c_sh])
            nc.sync.dma_start(rowacc[1], acc_erf[:, :c_sh])

        if k_full:
            # log(cond_E) and log(surv) for own full chunks; outputs into
            # out_scr slices keep the WAW chain so the Ln ops schedule
            # after the erf ops.  (the th term is assembled host-side)
            c0_t = acc.tile([P, 1], mybir.dt.float32)
            nc.vector.memset(c0_t[:], c0)
            half_t = acc.tile([P, 1], mybir.dt.float32)
            nc.vector.memset(half_t[:], half_eff)
            ln_ce = out_scr[:, 0:k_full]
            ln_sv = out_scr[:, k_full : 2 * k_full]
            nc.scalar.activation(
                ln_ce,
                acc_pdf[:, c_sh:],
                mybir.ActivationFunctionType.Ln,
                bias=c0_t[:],
                scale=k1,
            )
            nc.scalar.activation(
                ln_sv,
                acc_erf[:, c_sh:],
                mybir.ActivationFunctionType.Ln,
                bias=half_t[:],
                scale=k2,
            )

            # row_l[p] = sum_c (ln_ce - ln_sv); then -1/n * partition-sum
            t1 = acc.tile([P, k_full], mybir.dt.float32)
            nc.vector.tensor_sub(t1[:], ln_ce, ln_sv)
            row_l = acc.tile([P, 1], mybir.dt.float32)
            nc.vector.reduce_sum(row_l[:], t1[:], axis=mybir.AxisListType.X)
            tot = acc.tile([1, 1], mybir.dt.float32)
            partition_sum(tc, tot[:1, :1], row_l[:, :1])
            res = acc.tile([1, 1], mybir.dt.float32)
            nc.scalar.mul(res[:1, :1], tot[:1, :1], -1.0 / n)
            nc.sync.dma_start(partial[:], res[:1, :1])
        else:
            res = acc.tile([1, 1], mybir.dt.float32)
            nc.vector.memset(res[:1, :1], 0.0)
            nc.sync.dma_start(partial[:], res[:1, :1])

    nc.compile()
    return nc


def kernel(log_h: np.ndarray, durations: np.ndarray, events: np.ndarray) -> np.ndarray:
    global LAST_RESULTS

    theta = np.asarray(log_h).astype(np.float32, copy=False).reshape(-1)
    durations = np.asarray(durations).astype(np.float32, copy=False)
    events = np.asarray(events)
    n = int(theta.shape[0])

    e = -(theta - np.log(durations + np.float32(_EPS)))
    perm = np.argsort(e, kind="stable")
    e_sorted = np.ascontiguousarray(e[perm])
    inv = np.argsort(perm, kind="stable")
    ev = events.astype(np.float32)[inv]
    th_s = theta[inv]

    idx = np.nonzero(ev > 0.5)[0]
    n1 = int(idx.size)
    if n1 == 0:
        return np.array(-0.0, dtype=np.float32)

    e1 = e_sorted[idx]
    th1 = th_s[idx]

    # event cols padded to a multiple of 4*N_CORES (quarter-split and
    # narrow-split alignment; no need for full 128 alignment)
    ne = -(-n1 // (4 * N_CORES)) * (4 * N_CORES)
    na = -(-n // N_CORES) * N_CORES  # total cols padded to a multiple of 8
    k_full = n1 // (P * N_CORES)  # full 128-row chunks per core
    rem = n1 - k_full * P * N_CORES
    c_sh = -(-rem // P)  # shared chunks, columns split across cores
    nch = k_full + c_sh
    ne_nar = ne // N_CORES
    na_nar = na // N_CORES

    e_ev = np.full(ne, PAD_COL, dtype=np.float32)
    e_ev[:n1] = e1
    e_all = np.full(na, PAD_COL, dtype=np.float32)
    e_all[:n] = e_sorted

    # shared-chunk bias rows (identical on every core)
    shared_b = np.zeros(c_sh * P, dtype=np.float32)
    n_shared = rem
    if n_shared:
        shared_b[:n_shared] = e1[k_full * P * N_CORES :] * np.float32(RSQRT2)

    in_maps = []
    for c in range(N_CORES):
        s = c * k_full * P
        erb = np.empty(nch * P, dtype=np.float32)
        erb[: c_sh * P] = shared_b
        erb[c_sh * P :] = e1[s : s + k_full * P] * np.float32(RSQRT2)
        m = {"e_ev": e_ev, "e_all": e_all, "er_b": erb}
        if c_sh:
            m["e_ev_nar"] = np.ascontiguousarray(
                e_ev[c * ne_nar : (c + 1) * ne_nar]
            )
            m["e_all_nar"] = np.ascontiguousarray(
                e_all[c * na_nar : (c + 1) * na_nar]
            )
        in_maps.append(m)

    key = (k_full, c_sh, ne, na, n)
    if key not in _nc_cache:
        _nc_cache[key] = _build(*key)
    nc = _nc_cache[key]

    LAST_RESULTS = run_bass_kernel_spmd(
        nc, in_maps, core_ids=list(range(N_CORES)), trace=TRACE
    )

    loss = np.float64(0.0)
    for r in LAST_RESULTS.results:
        loss += float(r["partial"].reshape(()))
    # th term for the full-slot rows (device partials carry only the logs)
    loss += -np.float64(np.sum(th1[: k_full * P * N_CORES], dtype=np.float64)) / n

    if c_sh:
        # combine the shared chunks' per-core partial accumulators
        praw = np.zeros((P, c_sh), dtype=np.float64)
        eraw = np.zeros((P, c_sh), dtype=np.float64)
        for r in LAST_RESULTS.results:
            praw += r["rowacc"][0].astype(np.float64)
            eraw += r["rowacc"][1].astype(np.float64)
        praw = praw.T.reshape(-1)[:n_shared]  # rows are (c p)
        eraw = eraw.T.reshape(-1)[:n_shared]
        cond_e = praw / (2.0 * math.sqrt(2.0) * n) + n * _EPS
        surv = 0.5 + (eraw + (na - n)) / (2.0 * n)
        th_sh = th1[k_full * P * N_CORES :].astype(np.float64)
        loss += -np.sum(np.log(cond_e) - np.log(surv) + th_sh) / n

    return np.asarray(loss, dtype=np.float32)
